# revision 1
# baseline (speedup 1.0000x reference)
"""Trainium2 Bass kernel for nn_Attn (bahdanau-style attention scores).

Reference computation:
    energy = einsum('bsh,kh->bsk', encoder_outputs, W) + b    # [BS, S, H]
    scores = einsum('bsh,bh->bs', energy, hidden)             # [BS, S]
    out    = softmax(scores, axis=-1)

Algebraic restructuring used here:
    scores[b,s] = enc[b,s,:] . (hidden[b] @ W) + (hidden[b] . bias)
The bias term is constant along s, so it drops out of the softmax:
    out = softmax(enc[b] @ u[b]),   u = hidden @ W
This turns a 137-GFLOP problem into a DMA-bound streaming problem
(256 MB of encoder_outputs reads, ~32 MB and ~100 us per core).

Sharding: data-parallel over batch; core c handles batches [4c, 4c+4).
Per-core device pipeline:
  1. u = hidden_c @ W on the tensor engine.  W streams in as eight 512 KB
     chunks with the chunk matmuls pipelined behind the DMA so u is ready
     ~15 us into the kernel; enc streaming begins concurrently.
  2. broadcast u[b] rows to all 128 partitions (selector matmul, PSUM
     copies on the then-idle DVE).
  3. stream enc as eight 4 MB tiles [128 s-positions, 8x1024 h]; for each
     [128, 1024] chunk the DVE computes the elementwise product with the
     broadcast u and the scalar engine reduces it (activation-Copy with
     accum_out) into one score column.  The two engines split the
     elementwise work (~82 us and ~85 us) and hide under the DMA.
  4. per-batch partial maxes are appended as extra columns, the [128, 68]
     score block is transposed on the tensor engine and re-laid to
     [4, 2048] rows by one SBUF->SBUF DMA.
  5. row softmax: fused exp+sum on ACT (bias = -max), reciprocal,
     per-partition scale on DVE.
"""

import numpy as np

N_CORES = 8
BS, S, H = 32, 2048, 1024
BPC = BS // N_CORES          # batches per core
P = 128                      # partitions
KC = H // P                  # 8 contraction chunks for u
SG = S // 1024               # 2 s-groups of 1024 per batch
MT = BPC * SG                # 8 mega-tiles per core, each [128, 8*H] = 4 MB
SC = 1024 // P               # 8 s-chunks per mega-tile
NCOLS = MT * SC              # 64 score columns
XCOLS = NCOLS + BPC          # + one partial-max column per batch

# small-const pack free-dim offsets (hiddenT chunks | selector | identity)
OFF_HT = 0                   # [128, KC*BPC]
OFF_SEL = OFF_HT + KC * BPC  # [4, BPC*P]
OFF_ID = OFF_SEL + BPC * P   # [128, 128]
CONST_F = OFF_ID + P

_STATE = {}


def _build(loop_repeats=1):
    """Build the per-core Bass program.

    loop_repeats > 1 wraps the streaming + softmax body in a hardware
    For_i loop — used only for benchmarking (amortizes host dispatch
    overhead so per-iteration HW time can be measured from wall-clock).
    """
    import contextlib

    import concourse.bacc as bacc
    import concourse.mybir as mybir
    import concourse.tile as tile

    f32 = mybir.dt.float32
    # Bacc (not raw Bass): its lowering legalizes instructions that carry
    # more than one semaphore wait, which walrus codegen rejects.
    nc = bacc.Bacc(
        "TRN2", target_bir_lowering=False, debug=False, num_devices=N_CORES
    )

    enc = nc.dram_tensor("enc", [BPC, S, H], f32, kind="ExternalInput").ap()
    consts = nc.dram_tensor(
        "consts", [P, CONST_F], f32, kind="ExternalInput"
    ).ap()
    # W pre-chunked on host: wl[p, kc*H + h] = W[kc*128 + p, h]
    wl = nc.dram_tensor("wl", [P, KC * H], f32, kind="ExternalInput").ap()
    out = nc.dram_tensor("out", [BPC, S], f32, kind="ExternalOutput").ap()

    with tile.TileContext(nc) as tc:
        with (
            tc.tile_pool(name="const", bufs=1) as const_pool,
            tc.tile_pool(name="wpool", bufs=1) as wpool,
            tc.tile_pool(name="encp", bufs=3) as enc_pool,
            tc.tile_pool(name="scratch", bufs=3) as scratch_pool,
            tc.tile_pool(name="small", bufs=1) as small_pool,
            tc.tile_pool(name="ps1", bufs=1, space="PSUM") as ps1,
            tc.tile_pool(name="ps2", bufs=2, space="PSUM") as ps2,
            tc.tile_pool(name="dram", bufs=1, space="DRAM") as dram_pool,
        ):
            # ---- small consts first (one tiny DMA), then W in KC chunks so
            # the u matmuls pipeline behind the W transfer.
            c_sb = const_pool.tile([P, CONST_F], f32)
            nc.gpsimd.dma_start(c_sb[:], consts[:])
            ht_sb = c_sb[:, OFF_HT:OFF_HT + KC * BPC]
            sel_sb = c_sb[0:BPC, OFF_SEL:OFF_SEL + BPC * P]
            ident_sb = c_sb[:, OFF_ID:OFF_ID + P]

            w_sb = wpool.tile([P, KC * H], f32)          # 4 MB
            u_ps = [
                ps1.tile([BPC, 512], f32, tag=f"u_ps{i}", name=f"u_ps{i}")
                for i in range(2)
            ]
            for kc in range(KC):
                nc.gpsimd.dma_start(
                    w_sb[:, kc * H:(kc + 1) * H], wl[:, kc * H:(kc + 1) * H]
                )
                for nn in range(2):
                    nc.tensor.matmul(
                        u_ps[nn][:],
                        lhsT=ht_sb[:, kc * BPC:(kc + 1) * BPC],
                        rhs=w_sb[:, kc * H + nn * 512: kc * H + (nn + 1) * 512],
                        start=(kc == 0),
                        stop=(kc == KC - 1),
                    )
            u_sb = small_pool.tile([BPC, H], f32)
            for nn in range(2):
                nc.scalar.copy(u_sb[:, nn * 512:(nn + 1) * 512], u_ps[nn][:])

            # ---- broadcast u rows: u_bc[p, b*H + h] = u[b, h]
            # PSUM->SBUF copies ride the DVE, which is idle in the prefix.
            u_bc = const_pool.tile([P, BPC * H], f32)    # 2 MB
            for b in range(BPC):
                for nn in range(2):
                    bc_ps = ps2.tile([P, 512], f32, tag="bc_ps", name="bc_ps")
                    nc.tensor.matmul(
                        bc_ps[:],
                        lhsT=sel_sb[:, b * P:(b + 1) * P],
                        rhs=u_sb[:, nn * 512:(nn + 1) * 512],
                        start=True,
                        stop=True,
                    )
                    nc.vector.tensor_copy(
                        u_bc[:, b * H + nn * 512: b * H + (nn + 1) * 512],
                        bc_ps[:],
                    )

            # ---- main streaming loop
            loop_ctx = (
                tc.For_i(0, loop_repeats, 1) if loop_repeats > 1
                else contextlib.nullcontext()
            )
            with loop_ctx:
              sc_col = small_pool.tile([P, XCOLS], f32)
              for mt in range(MT):
                b, sg = divmod(mt, SG)
                et = enc_pool.tile([P, SC * H], f32)     # 4 MB
                # two 2 MB halves so the first s-chunks are consumable
                # while the second half is still in flight
                half = SC // 2
                for hv in range(2):
                    s0 = sg * 1024 + hv * half * P
                    nc.gpsimd.dma_start(
                        et[:, hv * half * H:(hv + 1) * half * H].rearrange(
                            "p (sc h) -> p sc h", h=H
                        ),
                        enc[b, s0:s0 + half * P, :].rearrange(
                            "(sc p) h -> p sc h", p=P
                        ),
                    )
                for sc in range(SC):
                    col = mt * SC + sc
                    # multiply on DVE; reduce on ACT (activation Copy with
                    # accum_out) so the two engines split the work.
                    pr = scratch_pool.tile([P, H], f32, tag="pr")
                    nc.vector.tensor_mul(
                        pr[:],
                        et[:, sc * H:(sc + 1) * H],
                        u_bc[:, b * H:(b + 1) * H],
                    )
                    pr2 = scratch_pool.tile([P, H], f32, tag="pr2")
                    nc.scalar.activation(
                        pr2[:],
                        pr[:],
                        mybir.ActivationFunctionType.Copy,
                        accum_out=sc_col[:, col:col + 1],
                    )

              # ---- per-batch partial max columns (over the 16 score columns
              # of each batch), appended so they ride the same transpose.
              for b in range(BPC):
                  nc.vector.reduce_max(
                      sc_col[:, NCOLS + b:NCOLS + b + 1],
                      sc_col[:, b * 16:(b + 1) * 16],
                      axis=mybir.AxisListType.X,
                  )

              # ---- transpose scores to row layout via PE + SBUF->SBUF DMA
              tp_ps = ps2.tile([XCOLS, P], f32, tag="tp_ps")
              nc.tensor.transpose(tp_ps[:], sc_col[:], ident_sb[:])
              scT = small_pool.tile([XCOLS, P], f32)
              nc.scalar.copy(scT[:], tp_ps[:])

              # Bounce through DRAM to regroup partitions into rows: DRAM holds
              # scT verbatim [68, 128]; reading rows b*16..b*16+16 contiguously
              # yields row b's 2048 scores.  SBUF-side APs stay plain (fancy
              # APs on SBUF reads break Tile's subtile dep tracking).
              sc_dram = dram_pool.tile([XCOLS, P], f32)
              nc.gpsimd.dma_start(sc_dram[:], scT[:])
              sc_row = small_pool.tile([BPC, S + P], f32)
              nc.gpsimd.dma_start(
                  sc_row[:, 0:S],
                  sc_dram[0:NCOLS, :].rearrange("(b g) f -> b (g f)", b=BPC),
              )
              nc.gpsimd.dma_start(sc_row[:, S:S + P], sc_dram[NCOLS:XCOLS, :])

              # ---- softmax over s
              rmax = small_pool.tile([BPC, 1], f32)
              nc.vector.reduce_max(
                  rmax[:], sc_row[:, S:S + P], axis=mybir.AxisListType.X
              )
              nmax = small_pool.tile([BPC, 1], f32)
              nc.vector.tensor_scalar_mul(nmax[:], rmax[:], -1.0)
              e_sb = small_pool.tile([BPC, S], f32)
              esum = small_pool.tile([BPC, 1], f32)
              nc.scalar.activation(
                  e_sb[:],
                  sc_row[:, 0:S],
                  mybir.ActivationFunctionType.Exp,
                  bias=nmax[:],
                  scale=1.0,
                  accum_out=esum[:],
              )
              rcp = small_pool.tile([BPC, 1], f32)
              nc.vector.reciprocal(rcp[:], esum[:])
              o_sb = small_pool.tile([BPC, S], f32)
              nc.vector.tensor_scalar_mul(o_sb[:], e_sb[:], rcp[:])
              nc.gpsimd.dma_start(out[:], o_sb[:])

    nc.compile()
    return nc


def _get_nc():
    if "nc" not in _STATE:
        _STATE["nc"] = _build()
    return _STATE["nc"]


def _make_in_maps(hidden, encoder_outputs, W):
    hidden = np.asarray(hidden, dtype=np.float32)
    encoder_outputs = np.asarray(encoder_outputs, dtype=np.float32)
    W = np.asarray(W, dtype=np.float32)

    # W laid out as [128, KC*H]: wl[p, kc*H + h] = W[kc*128 + p, h]
    wl = np.ascontiguousarray(
        W.reshape(KC, P, H).transpose(1, 0, 2).reshape(P, KC * H)
    )

    in_maps = []
    for c in range(N_CORES):
        hs = hidden[c * BPC:(c + 1) * BPC]          # [4, 1024]
        consts = np.zeros((P, CONST_F), dtype=np.float32)
        # htc[p, kc*BPC + b] = hs[b, kc*128 + p]
        consts[:, OFF_HT:OFF_HT + KC * BPC] = (
            hs.T.reshape(KC, P, BPC).transpose(1, 0, 2).reshape(P, KC * BPC)
        )
        for b in range(BPC):
            consts[b, OFF_SEL + b * P:OFF_SEL + (b + 1) * P] = 1.0
        consts[:, OFF_ID:OFF_ID + P] = np.eye(P, dtype=np.float32)
        in_maps.append(
            {
                "enc": np.ascontiguousarray(
                    encoder_outputs[c * BPC:(c + 1) * BPC]
                ),
                "consts": consts,
                "wl": wl,
            }
        )
    return in_maps


def run_sharded(hidden, encoder_outputs, W, trace=False, **trace_kwargs):
    from concourse.bass_utils import run_bass_kernel_spmd

    nc = _get_nc()
    in_maps = _make_in_maps(hidden, encoder_outputs, W)
    return run_bass_kernel_spmd(
        nc, in_maps, core_ids=list(range(N_CORES)), trace=trace, **trace_kwargs
    )


def kernel(hidden, encoder_outputs, W, b=None, **_ignored):
    res = run_sharded(hidden, encoder_outputs, W, trace=False)
    out = np.concatenate(
        [res.results[c]["out"] for c in range(N_CORES)], axis=0
    )
    return out.astype(np.float32)

